# revision 4
# baseline (speedup 1.0000x reference)
"""Trainium2 Bass kernel for nn_Actor (dense MLP trunk + 64 softmax heads).

Data-parallel over 8 NeuronCores: batch 4096 -> 512 rows/core, weights
replicated. Feature-major trunk (activations [features, batch]) so layer
outputs feed the next contraction without transposes; heads run batch-major
so per-head softmax reduces along the free dim.

Precision: trunk layers AND heads run fp8-e4m3 DoubleRow matmuls (256-deep
contraction per instruction; weights pre-scaled x256, x pre-scaled x32,
h2 stored as 32*h2 in fp8 -- all compensated via activation scale=).
Head bias is applied multiplicatively after the exp: softmax(l+b) =
exp(l)*exp(b)/sum, with eb=exp(b) replicated per-partition in SBUF and a
single [128, 640] vector multiply per head-pair.  This keeps the head
matmuls at exactly KTH=8 k-tiles (4 DR passes) instead of 10.

PE warm-up: ~12 dummy FD=512 matmuls on a zeroed SBUF tile run first so
the PE HAM clock-gate reaches 8/8 (2.4 GHz) before the first real matmul
(otherwise the whole L1 ramp runs at 1.2 GHz).

DMA: weights SBUF-resident; the L1 ramp (m0..m3, k-pair-major) gets its
w1 halves k-staged across sync (even m) and gpsimd (odd m) rings so
arrival order matches PE consumption order; xt streams on scalar.  The
last two k-pairs of the ramp run m-major so ACT's relu of m0 overlaps the
remaining ramp matmuls and m4 never waits on a PSUM bank.

Tail: the final batch-tile's softmax chain runs exp(scalar) -> eb-mul /
reduce / reciprocal / normalize all on vector (gpsimd's tensor ops are
~2.5x slower), with output DMAs on sync/gpsimd only -- never scalar, so
the final exp is not head-of-line blocked behind a waiting DMA issue.

Self-contained: hardcodes shapes; host-side prep packs head weights into one
[1024, 1280] fp8 GEMM whose columns are already in the final output order
(per vehicle v: rsu[2v] | rsu[2v+1] | lay[2v] | lay[2v+1]).
"""

import os
import numpy as np

B, IN_DIM, HIDDEN, H2 = 4096, 2048, 2048, 1024
V, R, L = 16, 32, 8
OUTC = V * (2 * R + 2 * L)          # 1280
NCORES = 8
BC = B // NCORES                    # 512 batch rows per core
KT1 = IN_DIM // 128                 # 16 k-tiles, layer 1
MT1 = HIDDEN // 128                 # 16 m-tiles, layer 1
KT2 = HIDDEN // 128                 # 16 k-tiles, layer 2
MT2 = H2 // 128                     # 8 m-tiles, layer 2
KTH = H2 // 128                     # 8 k-tiles, heads
BT = BC // 128                      # 4 batch tiles per core
CW = 320                            # head chunk width = 4 vehicles
NCH = OUTC // CW                    # 4 chunks
VC = CW // (2 * (R + L))            # 4 vehicles per chunk

_CACHE = {}
LAST_RESULTS = None                 # BassKernelResults from the last run


def _build():
    import concourse.bacc as bacc
    import concourse.mybir as mybir
    import concourse.tile as tile

    F32 = mybir.dt.float32
    F8 = mybir.dt.float8e4
    DR = mybir.MatmulPerfMode.DoubleRow
    Relu = mybir.ActivationFunctionType.Relu
    Exp = mybir.ActivationFunctionType.Exp
    X = mybir.AxisListType.X

    nc = bacc.Bacc("TRN2", target_bir_lowering=False, debug=False,
                   num_devices=NCORES)

    xt = nc.dram_tensor("xt", [128, KT1, BC], F8, kind="ExternalInput")
    w1t = nc.dram_tensor("w1t", [MT1, 128, KT1, 128], F8, kind="ExternalInput")
    b1c = nc.dram_tensor("b1c", [128, MT1], F32, kind="ExternalInput")
    w2t = nc.dram_tensor("w2t", [MT2, 128, KT2, 128], F8, kind="ExternalInput")
    b2c = nc.dram_tensor("b2c", [128, MT2], F32, kind="ExternalInput")
    wht = nc.dram_tensor("wht", [128, KTH, OUTC], F8, kind="ExternalInput")
    ebc = nc.dram_tensor("ebc", [128, OUTC], F32, kind="ExternalInput")
    out = nc.dram_tensor("out", [BC, OUTC], F32, kind="ExternalOutput")

    with tile.TileContext(nc) as tc:
        with (
            tc.tile_pool(name="const", bufs=1) as cp,
            tc.tile_pool(name="sm", bufs=6) as sp,
            tc.tile_pool(name="ps", bufs=4, space="PSUM") as ps,
            tc.tile_pool(name="psh", bufs=2, space="PSUM") as psh,
        ):
            xt_sb = cp.tile([128, KT1, BC], F8, tag="xt")
            h1_sb = cp.tile([128, KT2, BC], F8, tag="h1")
            h2_sb = cp.tile([128, KTH, BC], F8, tag="h2")
            wh_sb = cp.tile([128, KTH, OUTC], F8, tag="wh")
            eb_sb = cp.tile([128, OUTC], F32, tag="eb")
            b1_sb = cp.tile([128, MT1], F32, tag="b1")
            b2_sb = cp.tile([128, MT2], F32, tag="b2")
            w1_sb = [cp.tile([128, KT1, 128], F8, name=f"w1_{m}",
                             tag=f"w1_{m}") for m in range(MT1)]
            w2_sb = [cp.tile([128, KT2, 128], F8, name=f"w2_{m}",
                             tag=f"w2_{m}") for m in range(MT2)]
            warm_sb = cp.tile([128, 512], F8, tag="warm")

            # --- PE warm-up: dummy matmuls on zeros so the HAM clock gate
            # opens (4/8 -> 8/8) before the first data-dependent matmul.
            nc.vector.memset(warm_sb[:], 0.0)
            wps = ps.tile([128, BC], F32, tag="acc")
            for _ in range(12):
                nc.tensor.matmul(wps[:], warm_sb[:, 0:128], warm_sb[:],
                                 start=True, stop=True)

            # --- DMA descriptors, all issued up front on 3 rings ---
            # scalar: the xt stream (feeds the ramp's moving operand).
            nc.scalar.dma_start(xt_sb[:, 0:2, :], xt.ap()[:, 0:2, :])
            nc.scalar.dma_start(xt_sb[:, 2:4, :], xt.ap()[:, 2:4, :])
            nc.scalar.dma_start(xt_sb[:, 4:8, :], xt.ap()[:, 4:8, :])
            nc.scalar.dma_start(xt_sb[:, 8:12, :], xt.ap()[:, 8:12, :])
            nc.scalar.dma_start(xt_sb[:, 12:16, :], xt.ap()[:, 12:16, :])

            # sync: even ramp m-tiles in k-staged halves, then even m-major
            # tiles, even w2, first half of wh.
            for a, b in ((0, 4), (4, 8), (8, 16)):
                nc.sync.dma_start(w1_sb[0][:, a:b, :], w1t.ap()[0][:, a:b, :])
                nc.sync.dma_start(w1_sb[2][:, a:b, :], w1t.ap()[2][:, a:b, :])
            for m in range(4, MT1, 2):
                nc.sync.dma_start(w1_sb[m][:], w1t.ap()[m])
            for m in range(0, MT2, 2):
                nc.sync.dma_start(w2_sb[m][:], w2t.ap()[m])
            nc.sync.dma_start(wh_sb[:, 0:4, :], wht.ap()[:, 0:4, :])

            # gpsimd: odd mirror + tiny biases early + wh tail + eb last.
            nc.gpsimd.dma_start(w1_sb[1][:, 0:4, :], w1t.ap()[1][:, 0:4, :])
            nc.gpsimd.dma_start(w1_sb[3][:, 0:4, :], w1t.ap()[3][:, 0:4, :])
            nc.gpsimd.dma_start(b1_sb[:], b1c.ap())
            nc.gpsimd.dma_start(b2_sb[:], b2c.ap())
            for a, b in ((4, 8), (8, 16)):
                nc.gpsimd.dma_start(w1_sb[1][:, a:b, :], w1t.ap()[1][:, a:b, :])
                nc.gpsimd.dma_start(w1_sb[3][:, a:b, :], w1t.ap()[3][:, a:b, :])
            for m in range(5, MT1, 2):
                nc.gpsimd.dma_start(w1_sb[m][:], w1t.ap()[m])
            for m in range(1, MT2, 2):
                nc.gpsimd.dma_start(w2_sb[m][:], w2t.ap()[m])
            nc.gpsimd.dma_start(wh_sb[:, 4:8, :], wht.ap()[:, 4:8, :])
            nc.gpsimd.dma_start(eb_sb[:], ebc.ap())

            # --- Layer 1: h1[m] = relu(sum_k w1[k,m].T @ xt[k] + b1[m]) ---
            # Ramp m0..3 k-pair-major on 4 PSUM banks so PE consumption
            # tracks chunk arrivals; the last two k-pairs run m-major so
            # relu(m0) overlaps the ramp tail and m4 never waits on a bank.
            RM = 4
            raccs = [ps.tile([128, BC], F32, name=f"racc{i}", tag="acc")
                     for i in range(RM)]
            for k in range(0, KT1 - 4, 2):
                for mi in range(RM):
                    nc.tensor.matmul(raccs[mi][:], w1_sb[mi][:, k:k + 2, :],
                                     xt_sb[:, k:k + 2, :],
                                     start=(k == 0), stop=False, perf_mode=DR)
            for mi in range(RM):
                for k in range(KT1 - 4, KT1, 2):
                    nc.tensor.matmul(raccs[mi][:], w1_sb[mi][:, k:k + 2, :],
                                     xt_sb[:, k:k + 2, :],
                                     start=False, stop=(k == KT1 - 2),
                                     perf_mode=DR)
                nc.scalar.activation(h1_sb[:, mi, :], raccs[mi][:], Relu,
                                     bias=b1_sb[:, mi:mi + 1],
                                     scale=1.0 / 512.0)
            for m in range(RM, MT1):
                acc = ps.tile([128, BC], F32, tag="acc")
                for k in range(0, KT1, 2):
                    nc.tensor.matmul(acc[:], w1_sb[m][:, k:k + 2, :],
                                     xt_sb[:, k:k + 2, :],
                                     start=(k == 0), stop=(k == KT1 - 2),
                                     perf_mode=DR)
                nc.scalar.activation(h1_sb[:, m, :], acc[:], Relu,
                                     bias=b1_sb[:, m:m + 1], scale=1.0 / 512.0)

            # --- Layer 2: h2[m] = relu(sum_k w2[k,m].T @ h1[k] + b2[m]) ---
            for m in range(MT2):
                acc = ps.tile([128, BC], F32, tag="acc")
                for k in range(0, KT2, 2):
                    nc.tensor.matmul(acc[:], w2_sb[m][:, k:k + 2, :],
                                     h1_sb[:, k:k + 2, :],
                                     start=(k == 0), stop=(k == KT2 - 2),
                                     perf_mode=DR)
                nc.scalar.activation(h2_sb[:, m, :], acc[:], Relu,
                                     bias=b2_sb[:, m:m + 1], scale=1.0 / 128.0)

            # --- Heads: logits = h2.T @ wh in fp8 DoubleRow, then softmax
            # with multiplicative bias: etb = exp(l)*eb; o = etb / sum(etb).
            def reduces(et, w, sdst):
                # grouped softmax sums: rsu groups (32-wide) and lay groups
                # (8-wide) into sdst [128, 4*w*VC]
                PW = w * CW
                VP = w * VC
                nv = et[:, 0:PW].rearrange("p (v x) -> p v x", v=VP)
                rsu4 = nv[:, :, 0:2 * R].rearrange("p v (h c) -> p v h c", h=2)
                lay4 = nv[:, :, 2 * R:].rearrange("p v (h c) -> p v h c", h=2)
                s_r = sdst[:, 0:2 * VP].rearrange("p (v h) -> p v h", h=2)
                s_l = sdst[:, 2 * VP:4 * VP].rearrange(
                    "p (v h) -> p v h", h=2)
                nc.vector.reduce_sum(out=s_r.unsqueeze(3), in_=rsu4, axis=X)
                nc.vector.reduce_sum(out=s_l.unsqueeze(3), in_=lay4, axis=X)

            def norm(et, c0, w, rsrc, oeng, rmeng, lmeng):
                # normalize: rsu block on rmeng, lay block on lmeng
                PW = w * CW
                VP = w * VC
                nv = et[:, 0:PW].rearrange("p (v x) -> p v x", v=VP)
                rsu4 = nv[:, :, 0:2 * R].rearrange("p v (h c) -> p v h c", h=2)
                lay4 = nv[:, :, 2 * R:].rearrange("p v (h c) -> p v h c", h=2)
                o_sb = sp.tile([128, 2 * CW], F32, tag="o")
                ov = o_sb[:, 0:PW].rearrange("p (v x) -> p v x", v=VP)
                orsu = ov[:, :, 0:2 * R].rearrange("p v (h c) -> p v h c", h=2)
                olay = ov[:, :, 2 * R:].rearrange("p v (h c) -> p v h c", h=2)
                r_r = rsrc[:, 0:2 * VP].rearrange("p (v h) -> p v h", h=2)
                r_l = rsrc[:, 2 * VP:4 * VP].rearrange(
                    "p (v h) -> p v h", h=2)
                rmeng.tensor_mul(
                    orsu, rsu4,
                    r_r.unsqueeze(3).broadcast_to([128, VP, 2, R]))
                lmeng.tensor_mul(
                    olay, lay4,
                    r_l.unsqueeze(3).broadcast_to([128, VP, 2, L]))
                oeng.dma_start(out.ap()[bsl, c0:c0 + PW], o_sb[:, 0:PW])

            pidx = 0
            for bt in range(BT):
                bsl = slice(bt * 128, (bt + 1) * 128)
                last_bt = bt == BT - 1
                if not last_bt:
                    sums_bt = sp.tile([128, 64], F32, tag="sums")
                    rec_bt = sp.tile([128, 64], F32, tag="rec")
                for pr in range(NCH // 2):
                    accs = []
                    if pidx % 2 == 0:
                        for ci in range(2):
                            hacc = psh.tile([128, CW], F32, tag=f"hacc{ci}")
                            accs.append(hacc)
                    else:
                        # odd pairs borrow the (now idle) trunk PSUM banks so
                        # four pairs are in flight
                        for ci in range(2):
                            hacc = ps.tile([128, BC], F32, tag="acc")
                            accs.append(hacc[:, 0:CW])
                    for k in range(0, KTH, 2):
                        for ci in range(2):
                            c = 2 * pr + ci
                            nc.tensor.matmul(accs[ci][:],
                                             h2_sb[:, k:k + 2, bsl],
                                             wh_sb[:, k:k + 2,
                                                   c * CW:(c + 1) * CW],
                                             start=(k == 0),
                                             stop=(k == KTH - 2),
                                             perf_mode=DR)
                    c0 = 2 * pr * CW
                    if not last_bt:
                        et = sp.tile([128, 2 * CW], F32, tag="et")
                        etb = sp.tile([128, 2 * CW], F32, tag="etb")
                        for ci in range(2):
                            nc.scalar.activation(et[:, ci * CW:(ci + 1) * CW],
                                                 accs[ci][:], Exp,
                                                 scale=1.0 / 8192.0)
                        nc.vector.tensor_mul(etb[:], et[:],
                                             eb_sb[:, c0:c0 + 2 * CW])
                        reduces(etb, 2, sums_bt[:, pr * 32:pr * 32 + 32])
                        nc.vector.reciprocal(rec_bt[:, pr * 32:pr * 32 + 32],
                                             sums_bt[:, pr * 32:pr * 32 + 32])
                        norm(etb, c0, 2, rec_bt[:, pr * 32:pr * 32 + 32],
                             oeng=nc.sync, rmeng=nc.gpsimd, lmeng=nc.vector)
                    elif pr == 0:
                        et = sp.tile([128, 2 * CW], F32, tag="et")
                        etb = sp.tile([128, 2 * CW], F32, tag="etb")
                        for ci in range(2):
                            nc.scalar.activation(et[:, ci * CW:(ci + 1) * CW],
                                                 accs[ci][:], Exp,
                                                 scale=1.0 / 8192.0)
                        nc.vector.tensor_mul(etb[:], et[:],
                                             eb_sb[:, c0:c0 + 2 * CW])
                        sums0 = sp.tile([128, 64], F32, tag="sums")
                        rec0 = sp.tile([128, 64], F32, tag="rec")
                        reduces(etb, 2, sums0[:, 0:32])
                        nc.vector.reciprocal(rec0[:, 0:32], sums0[:, 0:32])
                        norm(etb, c0, 2, rec0[:, 0:32],
                             oeng=nc.sync, rmeng=nc.gpsimd, lmeng=nc.vector)
                    else:
                        # final pair chunk-wise, all-vector, for the
                        # shortest possible tail chain
                        sa = sp.tile([128, 64], F32, tag="sums")
                        ra = sp.tile([128, 64], F32, tag="rec")
                        eta = sp.tile([128, CW], F32, tag="eta")
                        etba = sp.tile([128, CW], F32, tag="etba")
                        nc.scalar.activation(eta[:], accs[0][:], Exp,
                                             scale=1.0 / 8192.0)
                        nc.vector.tensor_mul(etba[:], eta[:],
                                             eb_sb[:, c0:c0 + CW])
                        reduces(etba, 1, sa[:, 0:16])
                        nc.vector.reciprocal(ra[:, 0:16], sa[:, 0:16])
                        norm(etba, c0, 1, ra[:, 0:16],
                             oeng=nc.gpsimd, rmeng=nc.vector, lmeng=nc.vector)
                        etb_ = sp.tile([128, CW], F32, tag="etbb")
                        eb_t = sp.tile([128, CW], F32, tag="ebt")
                        nc.scalar.activation(eb_t[:], accs[1][:], Exp,
                                             scale=1.0 / 8192.0)
                        nc.vector.tensor_mul(etb_[:], eb_t[:],
                                             eb_sb[:, c0 + CW:c0 + 2 * CW])
                        reduces(etb_, 1, sa[:, 32:48])
                        nc.vector.reciprocal(ra[:, 32:48], sa[:, 32:48])
                        norm(etb_, c0 + CW, 1, ra[:, 32:48],
                             oeng=nc.sync, rmeng=nc.vector, lmeng=nc.vector)
                    pidx += 1

    nc.compile()
    return nc


def _prep_shared(w1, b1, w2, b2, w_rsu, b_rsu, w_lay, b_lay):
    import ml_dtypes
    f = np.float32
    f8 = ml_dtypes.float8_e4m3
    w1t = np.ascontiguousarray(
        np.clip(w1 * 256.0, -240, 240).astype(f8)
        .reshape(KT1, 128, MT1, 128).transpose(2, 1, 0, 3))
    w2t = np.ascontiguousarray(
        np.clip(w2 * 256.0, -240, 240).astype(f8)
        .reshape(KT2, 128, MT2, 128).transpose(2, 1, 0, 3))
    b1c = np.ascontiguousarray(16.0 * b1.reshape(MT1, 128).T, dtype=f)
    b2c = np.ascontiguousarray(32.0 * b2.reshape(MT2, 128).T, dtype=f)

    wh = np.empty((H2, OUTC), dtype=f)
    bh = np.empty((OUTC,), dtype=f)
    for v in range(V):
        c = 2 * (R + L) * v
        wh[:, c:c + R] = w_rsu[2 * v]
        wh[:, c + R:c + 2 * R] = w_rsu[2 * v + 1]
        wh[:, c + 2 * R:c + 2 * R + L] = w_lay[2 * v]
        wh[:, c + 2 * R + L:c + 2 * (R + L)] = w_lay[2 * v + 1]
        bh[c:c + R] = b_rsu[2 * v]
        bh[c + R:c + 2 * R] = b_rsu[2 * v + 1]
        bh[c + 2 * R:c + 2 * R + L] = b_lay[2 * v]
        bh[c + 2 * R + L:c + 2 * (R + L)] = b_lay[2 * v + 1]
    wht = np.ascontiguousarray(
        np.clip(wh * 256.0, -240, 240).astype(f8)
        .reshape(KTH, 128, OUTC).transpose(1, 0, 2))
    ebc = np.ascontiguousarray(
        np.broadcast_to(np.exp(bh, dtype=f), (128, OUTC)))
    return {"w1t": w1t, "b1c": b1c, "w2t": w2t, "b2c": b2c,
            "wht": wht, "ebc": ebc}


def kernel(x, w1, b1, w2, b2, w_rsu, b_rsu, w_lay, b_lay):
    global LAST_RESULTS
    import ml_dtypes
    from concourse.bass_utils import run_bass_kernel_spmd

    if "nc" not in _CACHE:
        _CACHE["nc"] = _build()
    nc = _CACHE["nc"]

    shared = _prep_shared(np.asarray(w1, np.float32), np.asarray(b1, np.float32),
                          np.asarray(w2, np.float32), np.asarray(b2, np.float32),
                          np.asarray(w_rsu, np.float32), np.asarray(b_rsu, np.float32),
                          np.asarray(w_lay, np.float32), np.asarray(b_lay, np.float32))

    # x [B, IN] -> per-core xt [128, KT1, BC] with [p, k, n] = x[core*BC+n, k*128+p]
    # fp8 e4m3 with x*32 so small values clear the subnormal range; the
    # combined 32*256 scale comes out in the L1 relu (scale=1/512 -> 16*h1)
    xt_full = np.clip(np.ascontiguousarray(np.asarray(x, np.float32).T) * 32.0,
                      -240, 240) \
        .astype(ml_dtypes.float8_e4m3).reshape(KT1, 128, B).transpose(1, 0, 2)
    in_maps = []
    for c in range(NCORES):
        m = dict(shared)
        m["xt"] = np.ascontiguousarray(xt_full[:, :, c * BC:(c + 1) * BC])
        in_maps.append(m)

    trace = os.environ.get("KERNEL_TRACE", "") == "1"
    LAST_RESULTS = run_bass_kernel_spmd(nc, in_maps, core_ids=list(range(NCORES)),
                                        trace=trace)
    return np.concatenate([r["out"] for r in LAST_RESULTS.results], axis=0)


# revision 7
# speedup vs baseline: 1.0714x; 1.0714x over previous
"""Trainium2 Bass kernel for nn_Actor (dense MLP trunk + 64 softmax heads).

Data-parallel over 8 NeuronCores: batch 4096 -> 512 rows/core, weights
replicated. Feature-major trunk (activations [features, batch]) so layer
outputs feed the next contraction without transposes; heads run batch-major
so per-head softmax reduces along the free dim.

Precision: trunk layers AND heads run fp8-e4m3 DoubleRow matmuls (256-deep
contraction per instruction; weights pre-scaled x256, x pre-scaled x32,
h2 stored as 32*h2 in fp8 -- all compensated via activation scale=).
Head bias rides in the contraction (k-tiles 8..9 are constants so the
extra DR pair adds 8192*b to every logit row).  Softmax post-processing
runs in bf16 (exp -> bf16, grouped reduces / reciprocal / normalize all
bf16 on DVE 2x mode); the output is written to DRAM in bf16 and upcast
to f32 on the host.

PE warm-up: ~12 dummy FD=512 matmuls on a zeroed SBUF tile run first so
the PE HAM clock-gate reaches 8/8 (2.4 GHz) before the first real matmul
(otherwise the whole L1 ramp runs at 1.2 GHz).

DMA: weights SBUF-resident; the L1 ramp (m0..m3, k-pair-major) gets its
w1 halves k-staged across sync (even m) and gpsimd (odd m) rings so
arrival order matches PE consumption order; xt + biases stream on scalar.
The last two k-pairs of the ramp run m-major so ACT's relu of m0 overlaps
the remaining ramp matmuls and m4 never waits on a PSUM bank.  All
constant memsets run on vector (idle early) so gpsimd's ring starts with
its first w1 transfer immediately.

Tail: the final batch-tile's last head-pair is processed chunk-wise with
all-vector normalize (gpsimd's tensor ops are ~2.5x slower) and output
DMAs on sync/gpsimd only -- never scalar, so the final exp is not
head-of-line blocked behind a waiting DMA issue.

Self-contained: hardcodes shapes; host-side prep packs head weights into one
[1024, 1280] fp8 GEMM whose columns are already in the final output order
(per vehicle v: rsu[2v] | rsu[2v+1] | lay[2v] | lay[2v+1]).
"""

import os
import numpy as np

B, IN_DIM, HIDDEN, H2 = 4096, 2048, 2048, 1024
V, R, L = 16, 32, 8
OUTC = V * (2 * R + 2 * L)          # 1280
NCORES = 8
BC = B // NCORES                    # 512 batch rows per core
KT1 = IN_DIM // 128                 # 16 k-tiles, layer 1
MT1 = HIDDEN // 128                 # 16 m-tiles, layer 1
KT2 = HIDDEN // 128                 # 16 k-tiles, layer 2
MT2 = H2 // 128                     # 8 m-tiles, layer 2
KTH = H2 // 128                     # 8 k-tiles, heads
BT = BC // 128                      # 4 batch tiles per core
CW = 320                            # head chunk width = 4 vehicles
NCH = OUTC // CW                    # 4 chunks
VC = CW // (2 * (R + L))            # 4 vehicles per chunk

_CACHE = {}
LAST_RESULTS = None                 # BassKernelResults from the last run


def _build():
    import concourse.bacc as bacc
    import concourse.mybir as mybir
    import concourse.tile as tile

    F32 = mybir.dt.float32
    BF16 = mybir.dt.bfloat16
    F8 = mybir.dt.float8e4
    DR = mybir.MatmulPerfMode.DoubleRow
    Relu = mybir.ActivationFunctionType.Relu
    Exp = mybir.ActivationFunctionType.Exp
    X = mybir.AxisListType.X

    nc = bacc.Bacc("TRN2", target_bir_lowering=False, debug=False,
                   num_devices=NCORES)

    xt = nc.dram_tensor("xt", [128, KT1, BC], F8, kind="ExternalInput")
    w1t = nc.dram_tensor("w1t", [MT1, 128, KT1, 128], F8, kind="ExternalInput")
    b1c = nc.dram_tensor("b1c", [128, MT1], F32, kind="ExternalInput")
    w2t = nc.dram_tensor("w2t", [MT2, 128, KT2, 128], F8, kind="ExternalInput")
    b2c = nc.dram_tensor("b2c", [128, MT2], F32, kind="ExternalInput")
    wht = nc.dram_tensor("wht", [128, KTH + 1, OUTC], F8,
                         kind="ExternalInput")
    out = nc.dram_tensor("out", [BC, OUTC], BF16, kind="ExternalOutput")

    with tile.TileContext(nc) as tc:
        with (
            tc.tile_pool(name="const", bufs=1) as cp,
            tc.tile_pool(name="sm", bufs=6) as sp,
            tc.tile_pool(name="ps", bufs=4, space="PSUM") as ps,
            tc.tile_pool(name="psh", bufs=2, space="PSUM") as psh,
        ):
            xt_sb = cp.tile([128, KT1, BC], F8, tag="xt")
            h1_sb = cp.tile([128, KT2, BC], F8, tag="h1")
            h2_sb = cp.tile([128, KTH + 2, BC], F8, tag="h2")
            wh_sb = cp.tile([128, KTH + 2, OUTC], F8, tag="wh")
            b1_sb = cp.tile([128, MT1], F32, tag="b1")
            b2_sb = cp.tile([128, MT2], F32, tag="b2")
            w1_sb = [cp.tile([128, KT1, 128], F8, name=f"w1_{m}",
                             tag=f"w1_{m}") for m in range(MT1)]
            w2_sb = [cp.tile([128, KT2, 128], F8, name=f"w2_{m}",
                             tag=f"w2_{m}") for m in range(MT2)]
            warm_sb = cp.tile([128, 512], F8, tag="warm")

            # --- PE warm-up: dummy matmuls on zeros so the HAM clock gate
            # opens (4/8 -> 8/8) before the first data-dependent matmul.
            nc.vector.memset(warm_sb[:], 0.0)
            wps = ps.tile([128, BC], F32, tag="acc")
            for _ in range(12):
                nc.tensor.matmul(wps[:], warm_sb[:, 0:128], warm_sb[:],
                                 start=True, stop=True)

            # bias-in-contraction constants (vector is idle early):
            # k-tiles 8..9 of h2 (partition 0 of k=8 holds 32.0, rest zero)
            # so the extra DR pair adds 32*256*bh = 8192*bh[c] per logit.
            nc.vector.memset(h2_sb[:, KTH:KTH + 2, :], 0.0)
            nc.vector.memset(h2_sb[0:1, KTH, :], 32.0)
            nc.vector.memset(wh_sb[:, KTH + 1, :], 0.0)

            # --- DMA descriptors, all issued up front on 3 rings ---
            # scalar: the xt stream (feeds the ramp), then tiny biases.
            nc.scalar.dma_start(xt_sb[:, 0:2, :], xt.ap()[:, 0:2, :])
            nc.scalar.dma_start(xt_sb[:, 2:4, :], xt.ap()[:, 2:4, :])
            nc.scalar.dma_start(xt_sb[:, 4:8, :], xt.ap()[:, 4:8, :])
            nc.scalar.dma_start(xt_sb[:, 8:12, :], xt.ap()[:, 8:12, :])
            nc.scalar.dma_start(xt_sb[:, 12:16, :], xt.ap()[:, 12:16, :])
            nc.scalar.dma_start(b1_sb[:], b1c.ap())
            nc.scalar.dma_start(b2_sb[:], b2c.ap())

            # sync: even ramp m-tiles in k-staged halves, then even m-major
            # tiles, even w2, first half of wh.
            for a, b in ((0, 4), (4, 8), (8, 16)):
                nc.sync.dma_start(w1_sb[0][:, a:b, :], w1t.ap()[0][:, a:b, :])
                nc.sync.dma_start(w1_sb[2][:, a:b, :], w1t.ap()[2][:, a:b, :])
            for m in range(4, MT1, 2):
                nc.sync.dma_start(w1_sb[m][:], w1t.ap()[m])
            for m in range(0, MT2, 2):
                nc.sync.dma_start(w2_sb[m][:], w2t.ap()[m])
            nc.sync.dma_start(wh_sb[:, 0:4, :], wht.ap()[:, 0:4, :])

            # gpsimd: odd mirror + wh tail.
            for a, b in ((0, 4), (4, 8), (8, 16)):
                nc.gpsimd.dma_start(w1_sb[1][:, a:b, :], w1t.ap()[1][:, a:b, :])
                nc.gpsimd.dma_start(w1_sb[3][:, a:b, :], w1t.ap()[3][:, a:b, :])
            for m in range(5, MT1, 2):
                nc.gpsimd.dma_start(w1_sb[m][:], w1t.ap()[m])
            for m in range(1, MT2, 2):
                nc.gpsimd.dma_start(w2_sb[m][:], w2t.ap()[m])
            nc.gpsimd.dma_start(wh_sb[:, 4:KTH + 1, :],
                                wht.ap()[:, 4:KTH + 1, :])

            # --- Layer 1: h1[m] = relu(sum_k w1[k,m].T @ xt[k] + b1[m]) ---
            # Ramp m0..3 k-pair-major on 4 PSUM banks so PE consumption
            # tracks chunk arrivals; the last two k-pairs run m-major so
            # relu(m0) overlaps the ramp tail and m4 never waits on a bank.
            RM = 4
            raccs = [ps.tile([128, BC], F32, name=f"racc{i}", tag="acc")
                     for i in range(RM)]
            for k in range(0, KT1 - 4, 2):
                for mi in range(RM):
                    nc.tensor.matmul(raccs[mi][:], w1_sb[mi][:, k:k + 2, :],
                                     xt_sb[:, k:k + 2, :],
                                     start=(k == 0), stop=False, perf_mode=DR)
            for mi in range(RM):
                for k in range(KT1 - 4, KT1, 2):
                    nc.tensor.matmul(raccs[mi][:], w1_sb[mi][:, k:k + 2, :],
                                     xt_sb[:, k:k + 2, :],
                                     start=False, stop=(k == KT1 - 2),
                                     perf_mode=DR)
                nc.scalar.activation(h1_sb[:, mi, :], raccs[mi][:], Relu,
                                     bias=b1_sb[:, mi:mi + 1],
                                     scale=1.0 / 512.0)
            for m in range(RM, MT1):
                acc = ps.tile([128, BC], F32, tag="acc")
                for k in range(0, KT1, 2):
                    nc.tensor.matmul(acc[:], w1_sb[m][:, k:k + 2, :],
                                     xt_sb[:, k:k + 2, :],
                                     start=(k == 0), stop=(k == KT1 - 2),
                                     perf_mode=DR)
                nc.scalar.activation(h1_sb[:, m, :], acc[:], Relu,
                                     bias=b1_sb[:, m:m + 1], scale=1.0 / 512.0)

            # --- Layer 2: h2[m] = relu(sum_k w2[k,m].T @ h1[k] + b2[m]) ---
            for m in range(MT2):
                acc = ps.tile([128, BC], F32, tag="acc")
                for k in range(0, KT2, 2):
                    nc.tensor.matmul(acc[:], w2_sb[m][:, k:k + 2, :],
                                     h1_sb[:, k:k + 2, :],
                                     start=(k == 0), stop=(k == KT2 - 2),
                                     perf_mode=DR)
                nc.scalar.activation(h2_sb[:, m, :], acc[:], Relu,
                                     bias=b2_sb[:, m:m + 1], scale=1.0 / 128.0)

            # --- Heads: logits = h2.T @ wh in fp8 DoubleRow (bias rides in
            # k-tiles 8..9), then softmax in bf16.
            def reduces(et, w, sdst):
                # grouped softmax sums: rsu groups (32-wide) and lay groups
                # (8-wide) into sdst [128, 4*w*VC]
                PW = w * CW
                VP = w * VC
                nv = et[:, 0:PW].rearrange("p (v x) -> p v x", v=VP)
                rsu4 = nv[:, :, 0:2 * R].rearrange("p v (h c) -> p v h c", h=2)
                lay4 = nv[:, :, 2 * R:].rearrange("p v (h c) -> p v h c", h=2)
                s_r = sdst[:, 0:2 * VP].rearrange("p (v h) -> p v h", h=2)
                s_l = sdst[:, 2 * VP:4 * VP].rearrange(
                    "p (v h) -> p v h", h=2)
                with nc.allow_low_precision(reason="bf16 softmax sums"):
                    nc.vector.reduce_sum(out=s_r.unsqueeze(3), in_=rsu4,
                                         axis=X)
                    nc.vector.reduce_sum(out=s_l.unsqueeze(3), in_=lay4,
                                         axis=X)

            def recip(dst, srcv):
                with nc.allow_low_precision(reason="bf16 softmax recip"):
                    nc.vector.reciprocal(dst, srcv)

            def norm(et, c0, w, rsrc, oeng, rmeng, lmeng):
                # normalize: rsu block on rmeng, lay block on lmeng
                PW = w * CW
                VP = w * VC
                nv = et[:, 0:PW].rearrange("p (v x) -> p v x", v=VP)
                rsu4 = nv[:, :, 0:2 * R].rearrange("p v (h c) -> p v h c", h=2)
                lay4 = nv[:, :, 2 * R:].rearrange("p v (h c) -> p v h c", h=2)
                o_sb = sp.tile([128, 2 * CW], BF16, tag="o")
                ov = o_sb[:, 0:PW].rearrange("p (v x) -> p v x", v=VP)
                orsu = ov[:, :, 0:2 * R].rearrange("p v (h c) -> p v h c", h=2)
                olay = ov[:, :, 2 * R:].rearrange("p v (h c) -> p v h c", h=2)
                r_r = rsrc[:, 0:2 * VP].rearrange("p (v h) -> p v h", h=2)
                r_l = rsrc[:, 2 * VP:4 * VP].rearrange(
                    "p (v h) -> p v h", h=2)
                rmeng.tensor_mul(
                    orsu, rsu4,
                    r_r.unsqueeze(3).broadcast_to([128, VP, 2, R]))
                lmeng.tensor_mul(
                    olay, lay4,
                    r_l.unsqueeze(3).broadcast_to([128, VP, 2, L]))
                oeng.dma_start(out.ap()[bsl, c0:c0 + PW], o_sb[:, 0:PW])

            pidx = 0
            for bt in range(BT):
                bsl = slice(bt * 128, (bt + 1) * 128)
                last_bt = bt == BT - 1
                if not last_bt:
                    sums_bt = sp.tile([128, 64], BF16, tag="sums")
                    rec_bt = sp.tile([128, 64], BF16, tag="rec")
                for pr in range(NCH // 2):
                    accs = []
                    if pidx % 2 == 0:
                        for ci in range(2):
                            hacc = psh.tile([128, CW], F32, tag=f"hacc{ci}")
                            accs.append(hacc)
                    else:
                        # odd pairs borrow the (now idle) trunk PSUM banks so
                        # four pairs are in flight
                        for ci in range(2):
                            hacc = ps.tile([128, BC], F32, tag="acc")
                            accs.append(hacc[:, 0:CW])
                    for k in range(0, KTH + 2, 2):
                        for ci in range(2):
                            c = 2 * pr + ci
                            nc.tensor.matmul(accs[ci][:],
                                             h2_sb[:, k:k + 2, bsl],
                                             wh_sb[:, k:k + 2,
                                                   c * CW:(c + 1) * CW],
                                             start=(k == 0), stop=(k == KTH),
                                             perf_mode=DR)
                    c0 = 2 * pr * CW
                    if not last_bt:
                        et = sp.tile([128, 2 * CW], BF16, tag="et")
                        for ci in range(2):
                            nc.scalar.activation(et[:, ci * CW:(ci + 1) * CW],
                                                 accs[ci][:], Exp,
                                                 scale=1.0 / 8192.0)
                        reduces(et, 2, sums_bt[:, pr * 32:pr * 32 + 32])
                        recip(rec_bt[:, pr * 32:pr * 32 + 32],
                                             sums_bt[:, pr * 32:pr * 32 + 32])
                        norm(et, c0, 2, rec_bt[:, pr * 32:pr * 32 + 32],
                             oeng=nc.sync, rmeng=nc.gpsimd, lmeng=nc.vector)
                    elif pr == 0:
                        et = sp.tile([128, 2 * CW], BF16, tag="et")
                        for ci in range(2):
                            nc.scalar.activation(et[:, ci * CW:(ci + 1) * CW],
                                                 accs[ci][:], Exp,
                                                 scale=1.0 / 8192.0)
                        sums0 = sp.tile([128, 64], BF16, tag="sums")
                        rec0 = sp.tile([128, 64], BF16, tag="rec")
                        reduces(et, 2, sums0[:, 0:32])
                        recip(rec0[:, 0:32], sums0[:, 0:32])
                        norm(et, c0, 2, rec0[:, 0:32],
                             oeng=nc.sync, rmeng=nc.gpsimd, lmeng=nc.vector)
                    else:
                        # final pair chunk-wise, all-vector, for the
                        # shortest possible tail chain
                        sa = sp.tile([128, 64], BF16, tag="sums")
                        ra = sp.tile([128, 64], BF16, tag="rec")
                        eta = sp.tile([128, CW], BF16, tag="eta")
                        nc.scalar.activation(eta[:], accs[0][:], Exp,
                                             scale=1.0 / 8192.0)
                        reduces(eta, 1, sa[:, 0:16])
                        recip(ra[:, 0:16], sa[:, 0:16])
                        norm(eta, c0, 1, ra[:, 0:16],
                             oeng=nc.gpsimd, rmeng=nc.vector, lmeng=nc.vector)
                        etb = sp.tile([128, CW], BF16, tag="etb")
                        nc.scalar.activation(etb[:], accs[1][:], Exp,
                                             scale=1.0 / 8192.0)
                        reduces(etb, 1, sa[:, 32:48])
                        recip(ra[:, 32:48], sa[:, 32:48])
                        norm(etb, c0 + CW, 1, ra[:, 32:48],
                             oeng=nc.sync, rmeng=nc.vector, lmeng=nc.vector)
                    pidx += 1

    nc.compile()
    return nc


def _prep_shared(w1, b1, w2, b2, w_rsu, b_rsu, w_lay, b_lay):
    import ml_dtypes
    f = np.float32
    f8 = ml_dtypes.float8_e4m3
    w1t = np.ascontiguousarray(
        np.clip(w1 * 256.0, -240, 240).astype(f8)
        .reshape(KT1, 128, MT1, 128).transpose(2, 1, 0, 3))
    w2t = np.ascontiguousarray(
        np.clip(w2 * 256.0, -240, 240).astype(f8)
        .reshape(KT2, 128, MT2, 128).transpose(2, 1, 0, 3))
    b1c = np.ascontiguousarray(16.0 * b1.reshape(MT1, 128).T, dtype=f)
    b2c = np.ascontiguousarray(32.0 * b2.reshape(MT2, 128).T, dtype=f)

    wh = np.empty((H2, OUTC), dtype=f)
    bh = np.empty((OUTC,), dtype=f)
    for v in range(V):
        c = 2 * (R + L) * v
        wh[:, c:c + R] = w_rsu[2 * v]
        wh[:, c + R:c + 2 * R] = w_rsu[2 * v + 1]
        wh[:, c + 2 * R:c + 2 * R + L] = w_lay[2 * v]
        wh[:, c + 2 * R + L:c + 2 * (R + L)] = w_lay[2 * v + 1]
        bh[c:c + R] = b_rsu[2 * v]
        bh[c + R:c + 2 * R] = b_rsu[2 * v + 1]
        bh[c + 2 * R:c + 2 * R + L] = b_lay[2 * v]
        bh[c + 2 * R + L:c + 2 * (R + L)] = b_lay[2 * v + 1]
    whx = np.zeros((KTH + 1, 128, OUTC), dtype=f)
    whx[0:KTH] = (wh * 256.0).reshape(KTH, 128, OUTC)
    whx[KTH, 0, :] = 256.0 * bh
    wht = np.ascontiguousarray(
        np.clip(whx, -240, 240).astype(f8).transpose(1, 0, 2))
    return {"w1t": w1t, "b1c": b1c, "w2t": w2t, "b2c": b2c,
            "wht": wht}


def kernel(x, w1, b1, w2, b2, w_rsu, b_rsu, w_lay, b_lay):
    global LAST_RESULTS
    import ml_dtypes
    from concourse.bass_utils import run_bass_kernel_spmd

    if "nc" not in _CACHE:
        _CACHE["nc"] = _build()
    nc = _CACHE["nc"]

    shared = _prep_shared(np.asarray(w1, np.float32), np.asarray(b1, np.float32),
                          np.asarray(w2, np.float32), np.asarray(b2, np.float32),
                          np.asarray(w_rsu, np.float32), np.asarray(b_rsu, np.float32),
                          np.asarray(w_lay, np.float32), np.asarray(b_lay, np.float32))

    # x [B, IN] -> per-core xt [128, KT1, BC] with [p, k, n] = x[core*BC+n, k*128+p]
    # fp8 e4m3 with x*32 so small values clear the subnormal range; the
    # combined 32*256 scale comes out in the L1 relu (scale=1/512 -> 16*h1)
    xt_full = np.clip(np.ascontiguousarray(np.asarray(x, np.float32).T) * 32.0,
                      -240, 240) \
        .astype(ml_dtypes.float8_e4m3).reshape(KT1, 128, B).transpose(1, 0, 2)
    in_maps = []
    for c in range(NCORES):
        m = dict(shared)
        m["xt"] = np.ascontiguousarray(xt_full[:, :, c * BC:(c + 1) * BC])
        in_maps.append(m)

    trace = os.environ.get("KERNEL_TRACE", "") == "1"
    LAST_RESULTS = run_bass_kernel_spmd(nc, in_maps, core_ids=list(range(NCORES)),
                                        trace=trace)
    return np.concatenate([r["out"] for r in LAST_RESULTS.results],
                          axis=0).astype(np.float32)
